# revision 4
# baseline (speedup 1.0000x reference)
"""FEDformer FourierCrossAttention kernel for 8 TRN2 NeuronCores — v2.

One head per core. Per-core pipeline (b processed in two halves of 16):
  DFT:   single-pass fp32r matmuls, be-major output (attn1-ready layout);
         K coefficient pair-tiles transposed on PE into mode-major km16.
  attn1: X^T = Kr^T[Qr|Qi] + Ki^T[-Qi|Qr] per b, fp32r matmuls.
  tanh:  complex tanh via tau/sin/cos form (ACT trig + DVE range reduction).
  attn2: Y = [Kr;Ki]^T [T;Tf] per b (fp16), after [T;Tf] partition assembly.
  W:     Z = Wr^T Y + Wi^T Yf per mode (fp16), Yf = [-Yi|Yr] lane-local.
  irfft: Z^T (PE transpose) then G-matmul; fp16 output carries 2^40 scale
         (W carries 2^16, G carries 2^24); host multiplies by 2^-40.

All host work is relayout/cast + constant (DFT matrix) construction.
"""
import numpy as np

import concourse.bass as bass
import concourse.tile as tile
from concourse import bacc, mybir
from concourse.bass_utils import run_bass_kernel_spmd

F32 = mybir.dt.float32
F32R = mybir.dt.float32r
F16 = mybir.dt.float16
AF = mybir.ActivationFunctionType
OP = mybir.AluOpType

B, L, H, E, O, M = 32, 1024, 8, 64, 64, 64
NCHUNK = 8
HB = 16             # batches per half
WSHIFT = 16
GSHIFT = 24
OUT_SCALE = np.float32(2.0 ** (-WSHIFT - GSHIFT))

PI = np.float64(np.pi)
PI_HI = np.float32(3.140625)
PI_MID = np.float32(PI - np.float64(np.float32(3.140625)))
PI_LO = np.float32(PI - np.float64(np.float32(3.140625)) - np.float64(PI_MID))
MAGIC = np.float32(1.5 * 2 ** 23)
RH_LIM = np.nextafter(np.float32(np.pi) - np.float32(np.pi / 2), np.float32(0))


def build(debug=False):
    nc = bacc.Bacc("TRN2", target_bir_lowering=False, debug=False, num_devices=8)

    # ---- per-core inputs ----
    xq_d = nc.dram_tensor("xq", (NCHUNK, 128, 2, B * E), F16, kind="ExternalInput")
    xk_d = nc.dram_tensor("xk", (NCHUNK, 128, 2, B * E), F16, kind="ExternalInput")
    f_d = nc.dram_tensor("f", (128, NCHUNK, 2, 2 * M), F16, kind="ExternalInput")
    w3_d = nc.dram_tensor("w3", (64, 2, M, O), F16, kind="ExternalInput")
    g_d = nc.dram_tensor("g", (2 * M, L), F16, kind="ExternalInput")
    idq_d = nc.dram_tensor("idq", (128, 128), F32, kind="ExternalInput")
    id16_d = nc.dram_tensor("id16", (64, 64), F16, kind="ExternalInput")
    out_d = nc.dram_tensor("out", (B, O, L), F16, kind="ExternalOutput")
    dbg = {}
    if debug:
        for nm, shp, dt_ in (("d_kt", (128, 8, 128), F32), ("d_qt", (128, 8, 128), F32),
                             ("d_km", (128, B, E), F16), ("d_t", (128, 8, 128), F16),
                             ("d_x", (128, 8, 128), F32),
                             ("d_tt", (128, 2, 8, 128), F16),
                             ("d_y", (64, HB, 2, M), F16), ("d_z", (O, HB, 2, M), F16),
                             ("d_zp", (128, HB, O), F16)):
            dbg[nm] = nc.dram_tensor(nm, shp, dt_, kind="ExternalOutput")

    with tile.TileContext(nc) as tc:
        from contextlib import ExitStack
        stack = ExitStack()
        with stack:
            consts = stack.enter_context(tc.tile_pool(name="consts", bufs=1))
            chunks = stack.enter_context(tc.tile_pool(name="chunks", bufs=6))
            coeff = stack.enter_context(tc.tile_pool(name="coeff", bufs=1))
            work = stack.enter_context(tc.tile_pool(name="work", bufs=1))
            tmp = stack.enter_context(tc.tile_pool(name="tmp", bufs=1))
            outs = stack.enter_context(tc.tile_pool(name="outs", bufs=4))
            ups = stack.enter_context(tc.tile_pool(name="ups", bufs=2, space="PSUM"))
            ops_ps = stack.enter_context(tc.tile_pool(name="ops_ps", bufs=2, space="PSUM"))

            # ---------- constants ----------
            f_t = consts.tile([128, NCHUNK, 2, 2 * M], F16, tag="f")
            w3_t = consts.tile([64, 2, M, O], F16, tag="w3")
            g_t = consts.tile([2 * M, L], F16, tag="g")
            idq_t = consts.tile([128, 128], F32, tag="idq")
            id16_t = consts.tile([64, 64], F16, tag="id16")
            halfpi = consts.tile([128, 1], F32, tag="halfpi")
            nc.vector.memset(halfpi[:], float(np.pi / 2))

            # per-half persistent tiles
            k_t = [coeff.tile([128, 8, 128], F32, tag=f"kt{hf}", name=f"k_t{hf}") for hf in range(2)]
            q_t = [coeff.tile([128, 8, 128], F32, tag=f"qt{hf}", name=f"q_t{hf}") for hf in range(2)]
            qf_t = [coeff.tile([128, 8, 128], F32, tag=f"qf{hf}", name=f"qf_t{hf}") for hf in range(2)]
            km16 = coeff.tile([128, B, E], F16, tag="km16")
            t_t = [work.tile([128, 8, 128], F16, tag=f"t{hf}", name=f"t_t{hf}") for hf in range(2)]
            tf_t = [work.tile([128, 8, 128], F16, tag=f"tf{hf}", name=f"tf_t{hf}") for hf in range(2)]
            tt_t = [work.tile([128, 2, 8, 128], F16, tag=f"tt{hf}", name=f"tt_t{hf}") for hf in range(2)]
            y3_t = [work.tile([64, HB, 2, M], F16, tag=f"y3{hf}", name=f"y3_t{hf}") for hf in range(2)]
            yf3_t = [work.tile([64, HB, 2, M], F16, tag=f"yf3{hf}", name=f"yf3_t{hf}") for hf in range(2)]
            z_t = [work.tile([O, HB, 2, M], F16, tag=f"z{hf}", name=f"z_t{hf}") for hf in range(2)]
            zp_t = [work.tile([128, HB, O], F16, tag=f"zp{hf}", name=f"zp_t{hf}") for hf in range(2)]

            def utile(name):
                return ups.tile([128, 8, 128], F32, tag="u", name=name)

            for hf in range(2):
                csl = slice(hf * HB * E, (hf + 1) * HB * E)
                # ---------- DFT (fp32r, mode-major: out[2m, be], 512-wide) ----------
                kmd = utile(f"kmd{hf}")
                qmd = utile(f"qmd{hf}")
                kmv = kmd[:].rearrange("p g m -> p (g m)")
                qmv = qmd[:].rearrange("p g m -> p (g m)")
                def dft_mms(xc, psv, c):
                    st, sp = c == 0, c == NCHUNK - 1
                    fh, fl = f_t[:, c, 0, :], f_t[:, c, 1, :]
                    passes = ((fh, 0, st, False), (fh, 1, False, False),
                              (fl, 0, False, sp))
                    for fm, hl, st_, sp_ in passes:
                        for gg in range(2):
                            nc.tensor.matmul(
                                psv[:, gg * 512:(gg + 1) * 512], fm,
                                xc[:, hl, gg * 512:(gg + 1) * 512],
                                start=st_, stop=sp_)

                if hf == 0:
                    for c in range(NCHUNK):
                        kc = chunks.tile([128, 2, HB * E], F16, tag="kc",
                                         name=f"kc{hf}_{c}", bufs=8)
                        qc = chunks.tile([128, 2, HB * E], F16, tag="qc",
                                         name=f"qc{hf}_{c}", bufs=8)
                        if c % 2 == 0:
                            nc.sync.dma_start(out=kc, in_=xk_d[c][:, :, csl])
                            nc.scalar.dma_start(out=qc, in_=xq_d[c][:, :, csl])
                        else:
                            nc.scalar.dma_start(out=kc, in_=xk_d[c][:, :, csl])
                            nc.sync.dma_start(out=qc, in_=xq_d[c][:, :, csl])
                        if c == 0:
                            nc.sync.dma_start(out=f_t, in_=f_d[:])
                            nc.scalar.dma_start(out=idq_t, in_=idq_d[:])
                        if c == 4:
                            nc.scalar.dma_start(out=w3_t, in_=w3_d[:])
                            nc.sync.dma_start(out=g_t, in_=g_d[:])
                            nc.scalar.dma_start(out=id16_t, in_=id16_d[:])
                        dft_mms(kc, kmv, c)
                        dft_mms(qc, qmv, c)
                else:
                    # h1: all k first, then all q — avoids PE head-of-line
                    # blocking on the qmd buffer (held until tanh-h0 reads xps0)
                    for c in range(NCHUNK):
                        kc = chunks.tile([128, 2, HB * E], F16, tag="kc",
                                         name=f"kc{hf}_{c}", bufs=8)
                        eng = nc.sync if c % 2 == 0 else nc.scalar
                        eng.dma_start(out=kc, in_=xk_d[c][:, :, csl])
                        dft_mms(kc, kmv, c)
                    for c in range(NCHUNK):
                        qc = chunks.tile([128, 2, HB * E], F16, tag="qc",
                                         name=f"qc{hf}_{c}", bufs=8)
                        eng = nc.scalar if c % 2 == 0 else nc.sync
                        eng.dma_start(out=qc, in_=xq_d[c][:, :, csl])
                        dft_mms(qc, qmv, c)

                # mode-major copies: SBUF staging (fp32r) + km16 (fp16)
                kms = coeff.tile([128, 1024], F32, tag="kms", name=f"kms{hf}", bufs=2)
                qms = coeff.tile([128, 1024], F32, tag="qms", name=f"qms{hf}", bufs=2)
                nc.vector.tensor_copy(kms[:, 0:512], kmv[:, 0:512])
                nc.scalar.copy(qms[:, 0:512], qmv[:, 0:512])
                nc.vector.tensor_copy(kms[:, 512:1024], kmv[:, 512:1024])
                nc.scalar.copy(qms[:, 512:1024], qmv[:, 512:1024])
                nc.scalar.copy(
                    km16[:, hf * HB:(hf + 1) * HB, :],
                    kmd[:].rearrange("p g (b e) -> p (g b) e", e=E))

                # ---------- be-major via full-128 transposes ----------
                kbt = utile(f"kbt{hf}")
                qbt = utile(f"qbt{hf}")
                for blk in range(8):
                    nc.tensor.transpose(kbt[:, blk, :],
                                        kms[:, blk * 128:(blk + 1) * 128],
                                        idq_t[:])
                    nc.tensor.transpose(qbt[:, blk, :],
                                        qms[:, blk * 128:(blk + 1) * 128],
                                        idq_t[:])
                nc.vector.tensor_copy(k_t[hf][:, 0:4, :], kbt[:, 0:4, :])
                nc.scalar.copy(q_t[hf][:, 0:4, :], qbt[:, 0:4, :])
                nc.vector.tensor_scalar_mul(qf_t[hf][:, 0:4, 0:64],
                                            qbt[:, 0:4, 64:128], -1.0)
                nc.vector.tensor_copy(qf_t[hf][:, 0:4, 64:128], qbt[:, 0:4, 0:64])
                nc.vector.tensor_copy(k_t[hf][:, 4:8, :], kbt[:, 4:8, :])
                nc.scalar.copy(q_t[hf][:, 4:8, :], qbt[:, 4:8, :])
                nc.vector.tensor_scalar_mul(qf_t[hf][:, 4:8, 0:64],
                                            qbt[:, 4:8, 64:128], -1.0)
                nc.vector.tensor_copy(qf_t[hf][:, 4:8, 64:128], qbt[:, 4:8, 0:64])

                # ---------- attn1 (fp32r) ----------
                xps = utile(f"xps{hf}")
                for g in range(8):
                    for par in range(2):
                        sl = slice(64 * par, 64 * par + 64)
                        nc.tensor.matmul(xps[sl, g, :], k_t[hf][sl, g, 0:64],
                                         q_t[hf][sl, g, :], start=True, stop=False)
                        nc.tensor.matmul(xps[sl, g, :], k_t[hf][sl, g, 64:128],
                                         qf_t[hf][sl, g, :], start=False, stop=True)

                # ---------- complex tanh (tau/sin/cos form) ----------
                av = xps[:, :, 0:64]
                bv = xps[:, :, 64:128]
                if debug and hf == 0:
                    dxs = work.tile([128, 8, 128], F32, tag="dxs", name="dxs")
                    nc.scalar.copy(dxs[:], xps[:])
                    nc.sync.dma_start(out=dbg["d_x"][:], in_=dxs[:])

                def ctt(n):
                    return tmp.tile([128, 512], F32, tag="ct", name=f"ct{hf}_{n}", bufs=8)

                def shp(t):
                    return t[:].rearrange("p (g m) -> p g m", m=64)
                ct_b = ctt("b")
                nc.scalar.copy(shp(ct_b), bv)
                ct_n = ctt("n")
                nc.vector.tensor_scalar(ct_n[:], ct_b[:], float(1.0 / PI), float(MAGIC),
                                        OP.mult, OP.add)
                nc.vector.tensor_scalar_sub(ct_n[:], ct_n[:], float(MAGIC))
                ct_rh = ctt("rh")
                nc.vector.cody_waite_cascade(ct_rh[:], ct_b[:], ct_n[:], float(PI_HI),
                                             float(PI_MID), float(PI_LO))
                nc.vector.tensor_scalar(ct_rh[:], ct_rh[:], -float(RH_LIM), float(RH_LIM),
                                        OP.max, OP.min)
                ct_tau = ctt("tau")
                nc.scalar.activation(shp(ct_tau), av, AF.Tanh)
                ct_s = ctt("s")
                nc.scalar.activation(ct_s[:], ct_rh[:], AF.Sin)
                ct_c = ctt("c")
                nc.scalar.activation(ct_c[:], ct_rh[:], AF.Sin, bias=halfpi[:])
                ct_s2 = ctt("s2")
                nc.vector.tensor_mul(ct_s2[:], ct_s[:], ct_s[:])
                ct_c2 = ctt("c2")
                nc.vector.tensor_mul(ct_c2[:], ct_c[:], ct_c[:])
                ct_sc = ctt("sc")
                nc.vector.tensor_mul(ct_sc[:], ct_s[:], ct_c[:])
                ct_t2 = ctt("t2")
                nc.vector.tensor_mul(ct_t2[:], ct_tau[:], ct_tau[:])
                ct_d = ctt("d")
                nc.vector.tensor_mul(ct_d[:], ct_t2[:], ct_s2[:])
                nc.vector.tensor_add(ct_d[:], ct_d[:], ct_c2[:])
                ct_r = ctt("r")
                nc.vector.reciprocal(ct_r[:], ct_d[:])
                nc.vector.tensor_scalar(ct_t2[:], ct_t2[:], -1.0, 1.0, OP.mult, OP.add)
                ct_u = ctt("u")
                nc.vector.tensor_mul(ct_u[:], ct_sc[:], ct_t2[:])
                nc.vector.tensor_mul(t_t[hf][:, :, 0:64], shp(ct_tau), shp(ct_r))
                nc.vector.tensor_mul(t_t[hf][:, :, 64:128], shp(ct_u), shp(ct_r))
                # Tf = [-Ti | Tr]
                nc.vector.tensor_scalar_mul(tf_t[hf][:, :, 0:64], t_t[hf][:, :, 64:128], -1.0)
                nc.vector.tensor_copy(tf_t[hf][:, :, 64:128], t_t[hf][:, :, 0:64])

                # ---------- assemble TT_b = [T_b ; Tf_b] ----------
                tt = tt_t[hf]
                nc.vector.tensor_copy(tt[0:64, 0, :, :], t_t[hf][0:64, :, :])
                nc.scalar.copy(tt[64:128, 1, :, :], tf_t[hf][64:128, :, :])
                nc.gpsimd.dma_start(out=tt[64:128, 0, :, :], in_=tf_t[hf][0:64, :, :])
                nc.gpsimd.dma_start(out=tt[0:64, 1, :, :], in_=t_t[hf][64:128, :, :])

            # ===== back half per hf: attn2 / W / ztrans / irfft =====
            for hf in range(2):
                tt = tt_t[hf]
                for jb in range(2):
                    yu = utile(f"yps{hf}_{jb}")
                    for jj in range(8):
                        j = jb * 8 + jj
                        b = hf * HB + j
                        par = j % 2
                        g = j // 2
                        nc.tensor.matmul(yu[0:64, jj, :], km16[:, b, :],
                                         tt[:, par, g, :], start=True, stop=True)
                    yv = yu[0:64, :, :].rearrange("p j (ri m) -> p j ri m", m=M)
                    dst = y3_t[hf][:, jb * 8:(jb + 1) * 8, :, :]
                    if jb == 0:
                        nc.vector.tensor_copy(dst, yv)
                    else:
                        nc.scalar.copy(dst, yv)
                nc.vector.tensor_scalar_mul(yf3_t[hf][:, :, 0, :], y3_t[hf][:, :, 1, :], -1.0)
                nc.vector.tensor_copy(yf3_t[hf][:, :, 1, :], y3_t[hf][:, :, 0, :])

                for xb in range(2):
                    zu = utile(f"zps{hf}_{xb}")
                    for xx in range(32):
                        x = xb * 32 + xx
                        csl2 = slice((xx % 4) * 32, (xx % 4) * 32 + 32)
                        nc.tensor.matmul(
                            zu[0:64, xx // 4, csl2], w3_t[:, 0, x, :],
                            y3_t[hf][:, :, :, x].rearrange("p j ri -> p (j ri)"),
                            start=True, stop=False)
                        nc.tensor.matmul(
                            zu[0:64, xx // 4, csl2], w3_t[:, 1, x, :],
                            yf3_t[hf][:, :, :, x].rearrange("p j ri -> p (j ri)"),
                            start=False, stop=True)
                    dst = z_t[hf][:, :, :, xb * 32:(xb + 1) * 32]
                    src_ = zu[0:64, :, :].rearrange("p g (xs j ri) -> p j ri (g xs)",
                                                    xs=4, ri=2)
                    if xb == 0:
                        nc.vector.tensor_copy(dst, src_)
                    else:
                        nc.scalar.copy(dst, src_)

                ztu = utile(f"zt{hf}")
                for j in range(HB):
                    zsl = slice((j % 4) * 32, (j % 4) * 32 + 32)
                    nc.tensor.transpose(
                        ztu[:, j // 4, zsl].bitcast(F16),
                        z_t[hf][:, j, :, :].rearrange("p ri m -> p (ri m)"),
                        id16_t[:])
                for gg in range(4):
                    cp = nc.vector.tensor_copy if gg % 2 == 0 else nc.scalar.copy
                    cp(zp_t[hf][:, gg * 4:(gg + 1) * 4, :].rearrange("p j o -> p (j o)"),
                       ztu[:, gg, :].bitcast(F16))

                for pr in range(HB // 2):
                    b0 = hf * HB + 2 * pr
                    opg = ops_ps.tile([128, 2, 512], F32, tag="opg", name=f"opg{hf}_{pr}")
                    lhs = zp_t[hf][:, 2 * pr:2 * pr + 2, :].rearrange("p b o -> p (b o)")
                    nc.tensor.matmul(opg[:, 0, :], lhs, g_t[:, 0:512], start=True, stop=True)
                    nc.tensor.matmul(opg[:, 1, :], lhs, g_t[:, 512:1024], start=True, stop=True)
                    ot = outs.tile([128, 1024], F16, tag="ot", name=f"ot{hf}_{pr}")
                    otv = ot[:].rearrange("p (lh l) -> p lh l", lh=2)
                    nc.vector.tensor_copy(otv[:, 0, :], opg[:, 0, :])
                    nc.scalar.copy(otv[:, 1, :], opg[:, 1, :])
                    nc.gpsimd.dma_start(out=out_d[b0:b0 + 2, :, :], in_=ot[:])

            if debug:
                for nm, t in (("d_kt", k_t[0]), ("d_qt", q_t[0]), ("d_km", km16),
                              ("d_t", t_t[0]), ("d_tt", tt_t[0]), ("d_y", y3_t[0]),
                              ("d_z", z_t[0]), ("d_zp", zp_t[0])):
                    nc.sync.dma_start(out=dbg[nm][:], in_=t[:])

    nc.compile()
    return nc


_NC_CACHE = None


def _get_nc():
    global _NC_CACHE
    if _NC_CACHE is None:
        _NC_CACHE = build()
    return _NC_CACHE


def _host_prep(q, k, Wr, Wi):
    l = np.arange(L, dtype=np.float64)[:, None]
    m = np.arange(M, dtype=np.float64)[None, :]
    ang = 2.0 * np.pi * l * m / L
    F0 = np.concatenate([np.cos(ang), -np.sin(ang)], axis=1).astype(np.float32)  # [L, 2M]
    Fh = F0.astype(np.float16)
    Fl = (F0 - Fh.astype(np.float32)).astype(np.float16)
    F = np.empty((128, NCHUNK, 2, 2 * M), np.float16)
    F[:, :, 0, :] = Fh.reshape(NCHUNK, 128, 2 * M).transpose(1, 0, 2)
    F[:, :, 1, :] = Fl.reshape(NCHUNK, 128, 2 * M).transpose(1, 0, 2)

    cm = np.full(M, 2.0); cm[0] = 1.0
    ang2 = 2.0 * np.pi * m.T * np.arange(L, dtype=np.float64)[None, :] / L
    SC = 2.0 ** GSHIFT / (L * 512.0 * 512.0)
    G = np.concatenate([
        cm[:, None] * np.cos(ang2) * SC,
        -cm[:, None] * np.sin(ang2) * SC,
    ], axis=0).astype(np.float32).astype(np.float16)  # [2M, L]

    idq = np.eye(128, dtype=np.float32)
    id16 = np.eye(64, dtype=np.float16)

    maps = []
    for h in range(H):
        def lay(x):
            xs = np.ascontiguousarray(x[:, :, h, :].transpose(1, 0, 2)).reshape(L, B * E)
            hi = xs.astype(np.float16)
            lo = (xs - hi.astype(np.float32)).astype(np.float16)
            out = np.empty((NCHUNK, 128, 2, B * E), np.float16)
            out[:, :, 0, :] = hi.reshape(NCHUNK, 128, B * E)
            out[:, :, 1, :] = lo.reshape(NCHUNK, 128, B * E)
            return out
        w3 = np.empty((64, 2, M, O), np.float32)
        w3[:, 0] = (Wr[h] * 2.0 ** WSHIFT).transpose(0, 2, 1)  # [e,o,x]->[e,x,o]
        w3[:, 1] = (Wi[h] * 2.0 ** WSHIFT).transpose(0, 2, 1)
        maps.append({
            "xq": lay(q), "xk": lay(k),
            "f": F, "w3": w3.astype(np.float16), "g": G,
            "idq": idq, "id16": id16,
        })
    return maps


def kernel(q, k, v, Wr, Wi, _trace=False):
    q = np.asarray(q, np.float32)
    k = np.asarray(k, np.float32)
    Wr = np.asarray(Wr, np.float32)
    Wi = np.asarray(Wi, np.float32)
    nc = _get_nc()
    maps = _host_prep(q, k, Wr, Wi)
    try:
        res = run_bass_kernel_spmd(nc, maps, core_ids=list(range(H)), trace=_trace)
    except ModuleNotFoundError:
        res = run_bass_kernel_spmd(nc, maps, core_ids=list(range(H)), trace=False)
    out = np.stack([res.results[h]["out"] for h in range(H)], axis=1)  # [B,H,O,L] f16
    if _trace:
        kernel.last_results = res
    return out.astype(np.float32) * OUT_SCALE
